# revision 1
# baseline (speedup 1.0000x reference)
"""MiniSTU Trainium2 kernel.

Reformulation (no FFT): per batch b,
    out = T @ (x @ Mp) + sgn ⊙ (T @ (sgn ⊙ (x @ Mm)))
where T is the lower-triangular block-Toeplitz matrix from phi and
sgn[l] = (-1)^l.  Polyphase split: with B_even = A+ + A-, B_odd = A+ - A-
(A+ = x@Mp, A- = sgn⊙(x@Mm)), even output rows need only (T@B_even)_even
and odd rows only (T@B_odd)_odd — half the convolution MACs.  The two
M=64 matmuls per Toeplitz block run concurrently in disjoint PE
column-groups via tile_position (0,0)/(0,64).

8 cores = batch(2) x output-quarter(4), no collectives; fp16 operands,
fp32 PSUM accumulation; two k-halves to fit SBUF.
"""

import numpy as np

B, L, D, O, K, P = 2, 2048, 512, 512, 16, 128
NB = L // P       # 16 l-blocks
KH = 2            # k halves
KPH = K // KH     # 8 filters per half
NOQ = 4           # o-quarters
OS = O // NOQ     # 128 per-core o slice
N_CORES = 8

_cache = {}


def _build_bass(reps=1):
    import contextlib
    import concourse.mybir as mybir
    import concourse.tile as tile
    from concourse import bacc

    dt = mybir.dt
    f16, f32 = dt.float16, dt.float32

    nc = bacc.Bacc("TRN2", target_bir_lowering=False, debug=False,
                   num_devices=N_CORES)

    xt_d = nc.dram_tensor("xt", [P, 4, L], f16, kind="ExternalInput")
    mx_d = nc.dram_tensor("mx", [P, 4, K * 2 * OS], f16, kind="ExternalInput")
    ph_d = nc.dram_tensor("ph", [KH, 4, P, 4 * KPH * P], f16, kind="ExternalInput")
    out_d = nc.dram_tensor("out", [P, NB * OS], f32, kind="ExternalOutput")

    CH = KPH * 2 * OS          # 2048 columns per k-half in mx/a
    with tile.TileContext(nc) as tc:
        with (
            tc.tile_pool(name="const", bufs=1) as cpool,
            tc.tile_pool(name="phpool", bufs=1) as phpool,
            tc.tile_pool(name="apool", bufs=1) as apool,
            tc.tile_pool(name="opool", bufs=1) as opool,
        ):
            xt = cpool.tile([P, 4, L], f16, tag="xt")
            mx = cpool.tile([P, 4, K * 2 * OS], f16, tag="mx")
            a_ev = apool.tile([P, NB, KPH * OS], f16, tag="aev")
            a_od = apool.tile([P, NB, KPH * OS], f16, tag="aod")
            outacc = opool.tile([P, NB, OS], f32, tag="outacc")

            for dc in range(4):
                nc.sync.dma_start(out=xt[:, dc, :], in_=xt_d[:, dc, :])
                nc.sync.dma_start(out=mx[:, dc, :], in_=mx_d[:, dc, :])

            loop_cm = (tc.For_i(0, reps, 1,
                                hint_engines=(mybir.EngineType.PE,
                                              mybir.EngineType.DVE))
                       if reps > 1 else contextlib.nullcontext())
            with loop_cm:
                _emit_body(nc, tc, mybir, f16, f32, xt, mx, ph_d, phpool,
                           a_ev, a_od, outacc, out_d)

    nc.compile()
    return nc


def _emit_body(nc, tc, mybir, f16, f32, xt, mx, ph_d, phpool,
               a_ev, a_od, outacc, out_d):
    CH = KPH * 2 * OS
    # even rows of out_d / odd rows, as [64, NB*OS] strided views
    od_even = out_d[:].rearrange("(h two) c -> two h c", two=2)[0]
    od_odd = out_d[:].rearrange("(h two) c -> two h c", two=2)[1]

    for kh in range(KH):
        ph = phpool.tile([P, NB * KPH * P], f16, tag="ph")
        for q in range(4):
            nc.sync.dma_start(out=ph[:, q * 4096:(q + 1) * 4096],
                              in_=ph_d[kh, q])

        # ---- stage 1: psum = [x@Mp | x@Mm] per l-tile, drained to
        # B_even = plus + sgn*minus, B_odd = plus - sgn*minus (fp16)
        with tc.tile_pool(name="ps1", bufs=2, space="PSUM") as ps1pool:
            for lt in range(NB):
                ps = ps1pool.tile([P, 2048], f32, tag="ps1")
                for dc in range(4):
                    for n in range(4):
                        c0 = kh * CH + n * 512
                        nc.tensor.matmul(
                            ps[:, n * 512:(n + 1) * 512],
                            xt[:, dc, lt * P:(lt + 1) * P],
                            mx[:, dc, c0:c0 + 512],
                            start=(dc == 0), stop=(dc == 3),
                        )
                psv = ps[:].rearrange("p (a s o) -> p a s o", a=KPH, s=2, o=OS)
                avE = a_ev[:, lt, :].rearrange("p (a o) -> p a o", a=KPH, o=OS)
                avO = a_od[:, lt, :].rearrange("p (a o) -> p a o", a=KPH, o=OS)
                nc.vector.tensor_copy(avE[0:64], psv[0:64, :, 0, :])
                nc.vector.tensor_copy(avE[64:128], psv[64:128, :, 1, :])
                nc.vector.tensor_copy(avO[0:64], psv[0:64, :, 1, :])
                nc.vector.tensor_copy(avO[64:128], psv[64:128, :, 0, :])

        # ---- stage 2: even/odd polyphase conv, (d,kl)-outer, M=64
        # col-tiled parity pairs; aligned J-runs fused into up-to-N=512
        # MMs (accumulator quad (4q..4q+3) fills one PSUM bank; the quad
        # group stops at d == 4q+3 where only I=d's contribution exists).
        # 4 quads x 2 parities = 8 banks -> single pass per k-half.
        with tc.tile_pool(name="ps2", bufs=1, space="PSUM") as ps2pool:
            if True:
                ps2e = ps2pool.tile([P, 4, 512], f32, tag="ps2e")
                ps2o = ps2pool.tile([P, 4, 512], f32, tag="ps2o")
                for d in range(NB):
                    j_hi = NB - d
                    segs = []
                    J = 0
                    while J < j_hi:
                        w = min(4 - ((J + d) % 4), j_hi - J)
                        segs.append((J, w))
                        J += w
                    for kl in range(KPH):
                        blk = (d * KPH + kl) * P
                        for (J0, w) in segs:
                            I0 = J0 + d
                            q = I0 // 4
                            off = (I0 % 4) * OS
                            st = (d == 0 and kl == 0)
                            sp = (J0 == 0 and d % 4 == 3 and kl == KPH - 1)
                            nc.tensor.matmul(
                                ps2e[0:64, q, off:off + w * OS],
                                ph[:, blk:blk + 64],
                                a_ev[:, J0:J0 + w, kl * OS:(kl + 1) * OS],
                                start=st, stop=sp, tile_position=(0, 0),
                            )
                            nc.tensor.matmul(
                                ps2o[64:128, q, off:off + w * OS],
                                ph[:, blk + 64:blk + P],
                                a_od[:, J0:J0 + w, kl * OS:(kl + 1) * OS],
                                start=st, stop=sp, tile_position=(0, 64),
                            )
                    # quad q = d//4 closes after d == 4q+3
                    if d % 4 == 3:
                        q = d // 4
                        Ia = 4 * q
                        pse = ps2e[0:64, q, :].rearrange(
                            "p (i o) -> p i o", i=4, o=OS)
                        pso = ps2o[64:128, q, :].rearrange(
                            "p (i o) -> p i o", i=4, o=OS)
                        if kh == 0:
                            nc.vector.tensor_copy(
                                outacc[0:64, Ia:Ia + 4, :], pse)
                            nc.vector.tensor_copy(
                                outacc[64:128, Ia:Ia + 4, :], pso)
                        else:
                            nc.vector.tensor_add(
                                outacc[0:64, Ia:Ia + 4, :],
                                outacc[0:64, Ia:Ia + 4, :], pse)
                            nc.vector.tensor_add(
                                outacc[64:128, Ia:Ia + 4, :],
                                outacc[64:128, Ia:Ia + 4, :], pso)
                            nc.sync.dma_start(
                                out=od_even[:, Ia * OS:(Ia + 4) * OS],
                                in_=outacc[0:64, Ia:Ia + 4, :])
                            nc.sync.dma_start(
                                out=od_odd[:, Ia * OS:(Ia + 4) * OS],
                                in_=outacc[64:128, Ia:Ia + 4, :])


def _prep_inputs(x, phi, M_phi_plus, M_phi_minus):
    """Host-side shard prep. Returns list of 8 input dicts (cores = b*4 + oq).

    All sign handling is done here: s=0 carries Msum=Mp+Mm, s=1 Mdif=Mp-Mm,
    and l-rows are parity-permuted (even rows first within each 128-block),
    so B_even/B_odd on device are plain partition-range copies."""
    perm = np.concatenate([2 * np.arange(64), 2 * np.arange(64) + 1])  # [128]

    # xt[p, dc, lt*128 + q] = x[b, lt*128 + perm[q], dc*128+p]
    xts = []
    for b in range(B):
        xb = x[b].reshape(NB, P, D)[:, perm, :].reshape(L, D)
        xt = np.ascontiguousarray(
            xb.T.reshape(4, P, L).transpose(1, 0, 2)).astype(np.float16)
        xts.append(xt)

    # mx[p, dc, k*256 + s*128 + oo] = M_s[k, dc*128+p, oq*128+oo]
    mcat = np.stack([M_phi_plus + M_phi_minus,
                     M_phi_plus - M_phi_minus], axis=1)  # [K, 2, D, O]
    mxs = []
    for oq in range(NOQ):
        m = mcat[:, :, :, oq * OS:(oq + 1) * OS]        # [K, 2, D, OS]
        m = m.transpose(2, 0, 1, 3).reshape(D, K * 2 * OS)
        mx = np.ascontiguousarray(
            m.reshape(4, P, K * 2 * OS).transpose(1, 0, 2)).astype(np.float16)
        mxs.append(mx)

    # parity-split Toeplitz blocks: for block (d, k), column m of the
    # even half is output row 2m, of the odd half row 2m+1:
    #   ph[.., (dq, kl, par, m)] = phi[d*P + (2m+par) - pp, kh*KPH+kl]
    # contraction rows (pp) use the same parity permutation as xt's l-rows
    pcol = np.concatenate([2 * np.arange(64), 2 * np.arange(64) + 1])  # [128]
    diff = pcol[None, :] - pcol[:, None]                # [pp', m'] = p - pp
    v = np.arange(NB)[:, None, None] * P + diff[None]   # [d, pp, m']
    valid = v >= 0
    phb = np.zeros((NB, P, P, K), dtype=np.float32)     # [d, pp, m', k]
    phb[valid] = phi[v[valid], :]
    # [d, pp, m', (kh, kl)] -> [kh, q, pp, dq, kl, m']
    phb = phb.reshape(4, 4, P, P, KH, KPH).transpose(4, 0, 2, 1, 5, 3)
    ph = np.ascontiguousarray(phb.reshape(KH, 4, P, 4 * KPH * P)).astype(np.float16)

    in_maps = []
    for b in range(B):
        for oq in range(NOQ):
            in_maps.append({"xt": xts[b], "mx": mxs[oq], "ph": ph})
    return in_maps


def kernel(x, phi, M_phi_plus, M_phi_minus):
    from concourse.bass_utils import run_bass_kernel_spmd

    x = np.asarray(x, dtype=np.float32)
    phi = np.asarray(phi, dtype=np.float32)
    M_phi_plus = np.asarray(M_phi_plus, dtype=np.float32)
    M_phi_minus = np.asarray(M_phi_minus, dtype=np.float32)

    if "nc" not in _cache:
        _cache["nc"] = _build_bass()
    nc = _cache["nc"]

    in_maps = _prep_inputs(x, phi, M_phi_plus, M_phi_minus)
    results = run_bass_kernel_spmd(nc, in_maps, core_ids=list(range(N_CORES)))

    out = np.empty((B, L, O), dtype=np.float32)
    for c in range(N_CORES):
        b, oq = divmod(c, NOQ)
        r = results.results[c]["out"]                   # [P, NB*OS]
        blk = r.reshape(P, NB, OS).transpose(1, 0, 2).reshape(L, OS)
        out[b, :, oq * OS:(oq + 1) * OS] = blk
    return out



# revision 3
# speedup vs baseline: 1.1027x; 1.1027x over previous
"""MiniSTU Trainium2 kernel — low-rank far-field formulation.

out = T @ (x @ Mp) + sgn (T @ (sgn (x @ Mm))), T block-lower-triangular
Toeplitz from phi.  Polyphase: even output rows need (T@C)_even, odd rows
(T@D)_odd with C/D = x @ (Mp±Mm) interleaved by row parity (stage 1).

Stage 2 splits into:
  - d0: exact dense within-block conv (block distance 0), per filter.
  - far field (block distance d>=1): all 15 block matrices, jointly over
    all filters, share a common rank-R right-singular basis W per output
    parity (numerically R=16 captures 1e-4).  So: Y[J] = W^T B_J (one
    projection per l-block, k-contraction via PSUM accumulation), then
    out_I += sum_d U_d @ Y[I-d] with tiny rank-R matmuls.

This cuts stage-2 PE work ~3.4x vs dense block conv.  8 cores =
batch(2) x output-quarter(4), no collectives; fp16 operands, fp32 PSUM.
"""

import numpy as np

B, L, D, O, K, P = 2, 2048, 512, 512, 16, 128
K_USE = 16        # filters kept (largest sigma); 12 passes at rel err 1.49e-2
R = 32            # shared far-field basis rank per parity (<=32 for tile_position)
NB = L // P       # 16 l-blocks
KH = 2            # k groups (SBUF halving)
KPH = K_USE // KH
NOQ = 4           # o-quarters
OS = O // NOQ     # 128 per-core o slice
CH = KPH * 2 * OS
N_CORES = 8

_cache = {}


def _build_bass(reps=1):
    import contextlib
    import concourse.mybir as mybir
    import concourse.tile as tile
    from concourse import bacc

    dt = mybir.dt
    f16, f32 = dt.float16, dt.float32

    nc = bacc.Bacc("TRN2", target_bir_lowering=False, debug=False,
                   num_devices=N_CORES)

    xt_d = nc.dram_tensor("xt", [P, 4, L], f16, kind="ExternalInput")
    mx_d = nc.dram_tensor("mx", [P, 4, K_USE * 2 * OS], f16, kind="ExternalInput")
    t0_d = nc.dram_tensor("t0", [P, K_USE * P], f16, kind="ExternalInput")
    w_d = nc.dram_tensor("w", [P, K_USE * 2 * R], f16, kind="ExternalInput")
    u_d = nc.dram_tensor("u", [P, (NB - 1) * 64], f16, kind="ExternalInput")
    out_d = nc.dram_tensor("out", [P, NB * OS], f32, kind="ExternalOutput")

    with tile.TileContext(nc) as tc:
        with (
            tc.tile_pool(name="const", bufs=1) as cpool,
            tc.tile_pool(name="apool", bufs=1) as apool,
            tc.tile_pool(name="ypool", bufs=1) as ypool,
            tc.tile_pool(name="opool", bufs=1) as opool,
        ):
            xt = cpool.tile([P, 4, L], f16, tag="xt")
            mx = cpool.tile([P, 4, K_USE * 2 * OS], f16, tag="mx")
            t0 = cpool.tile([P, K_USE * P], f16, tag="t0")
            w = cpool.tile([P, K_USE * 2 * R], f16, tag="w")
            u = cpool.tile([P, (NB - 1) * 64], f16, tag="u")
            a_ev = apool.tile([P, NB, KPH * OS], f16, tag="aev")
            a_od = apool.tile([P, NB, KPH * OS], f16, tag="aod")
            ysb0 = ypool.tile([P, NB * OS], f16, tag="ysb0")
            ysb1 = ypool.tile([P, NB * OS], f16, tag="ysb1")
            ysum = ypool.tile([P, NB * OS], f16, tag="ysum")
            outacc = opool.tile([P, NB, OS], f32, tag="outacc")

            for dc in range(4):
                nc.sync.dma_start(out=xt[:, dc, :], in_=xt_d[:, dc, :])
                nc.sync.dma_start(out=mx[:, dc, :], in_=mx_d[:, dc, :])
            nc.sync.dma_start(out=t0[:], in_=t0_d[:])
            nc.sync.dma_start(out=w[:], in_=w_d[:])
            nc.sync.dma_start(out=u[:], in_=u_d[:])

            loop_cm = (tc.For_i(0, reps, 1,
                                hint_engines=(mybir.EngineType.PE,
                                              mybir.EngineType.DVE))
                       if reps > 1 else contextlib.nullcontext())
            with loop_cm:
                _emit_body(nc, tc, mybir, f16, f32, xt, mx, t0, w, u,
                           a_ev, a_od, ysb0, ysb1, ysum, outacc, out_d)

    nc.compile()
    return nc


def _emit_body(nc, tc, mybir, f16, f32, xt, mx, t0, w, u,
               a_ev, a_od, ysb0, ysb1, ysum, outacc, out_d):
    od_even = out_d[:].rearrange("(h two) c -> two h c", two=2)[0]
    od_odd = out_d[:].rearrange("(h two) c -> two h c", two=2)[1]

    for kh in range(KH):
        # ---- stage 1: psum = x @ [Msum|Mdif] per l-block, drained to
        # parity-interleaved C (a_ev) / D (a_od) rows in fp16.
        with tc.tile_pool(name="ps1", bufs=2, space="PSUM") as ps1pool:
            for J in range(NB):
                ps = ps1pool.tile([P, CH], f32, tag="ps1")
                for dc in range(4):
                    for n in range(CH // 512):
                        nc.tensor.matmul(
                            ps[:, n * 512:(n + 1) * 512],
                            xt[:, dc, J * P:(J + 1) * P],
                            mx[:, dc, kh * CH + n * 512: kh * CH + (n + 1) * 512],
                            start=(dc == 0), stop=(dc == 3),
                        )
                psv = ps[:].rearrange("p (a s o) -> p a s o", a=KPH, s=2, o=OS)
                avE = a_ev[:, J, :].rearrange("p (a o) -> p a o", a=KPH, o=OS)
                avO = a_od[:, J, :].rearrange("p (a o) -> p a o", a=KPH, o=OS)
                nc.vector.tensor_copy(avE[0:64], psv[0:64, :, 0, :])
                nc.vector.tensor_copy(avE[64:128], psv[64:128, :, 1, :])
                nc.vector.tensor_copy(avO[0:64], psv[0:64, :, 1, :])
                nc.vector.tensor_copy(avO[64:128], psv[64:128, :, 0, :])

        # ---- stage 2: pso = d0 (exact diagonal blocks) [+ far field at
        # kh=1]; psy = rank-R far-field projections Y[J] = W^T B_J.
        with (
            tc.tile_pool(name="ps2o", bufs=1, space="PSUM") as psopool,
            tc.tile_pool(name="ps2y", bufs=1, space="PSUM") as psypool,
        ):
            pso = psopool.tile([P, 4, 512], f32, tag="pso")
            psy = psypool.tile([P, 4, 512], f32, tag="psy")

            for kl in range(KPH):
                tc0 = (kh * KPH + kl) * P
                for q in range(4):
                    st = (kl == 0)
                    sp = (kh == 0 and kl == KPH - 1)
                    nc.tensor.matmul(
                        pso[0:64, q, :],
                        t0[:, tc0:tc0 + 64],
                        a_ev[:, 4 * q:4 * q + 4, kl * OS:(kl + 1) * OS],
                        start=st, stop=sp, tile_position=(0, 0),
                    )
                    nc.tensor.matmul(
                        pso[64:128, q, :],
                        t0[:, tc0 + 64:tc0 + P],
                        a_od[:, 4 * q:4 * q + 4, kl * OS:(kl + 1) * OS],
                        start=st, stop=sp, tile_position=(0, 64),
                    )

            for kl in range(KPH):
                wc = (kh * KPH + kl) * 2 * R
                for q in range(4):
                    jn = 4 if q < 3 else 3      # Y[15] is never used
                    st = (kl == 0)
                    sp = (kl == KPH - 1)
                    nc.tensor.matmul(
                        psy[0:R, q, 0:jn * OS],
                        w[:, wc:wc + R],
                        a_ev[:, 4 * q:4 * q + jn, kl * OS:(kl + 1) * OS],
                        start=st, stop=sp, tile_position=(0, 0),
                    )
                    nc.tensor.matmul(
                        psy[64:64 + R, q, 0:jn * OS],
                        w[:, wc + R:wc + 2 * R],
                        a_od[:, 4 * q:4 * q + jn, kl * OS:(kl + 1) * OS],
                        start=st, stop=sp, tile_position=(0, 64),
                    )

            ysb = ysb0 if kh == 0 else ysb1
            for q in range(4):
                jn = 4 if q < 3 else 3
                c0, cw = 4 * q * OS, jn * OS
                nc.vector.tensor_copy(ysb[0:R, c0:c0 + cw], psy[0:R, q, 0:cw])
                nc.vector.tensor_copy(ysb[64:64 + R, c0:c0 + cw],
                                      psy[64:64 + R, q, 0:cw])

            if kh == KH - 1:
                cw = (NB - 1) * OS
                nc.vector.tensor_add(ysum[0:R, 0:cw], ysb0[0:R, 0:cw],
                                     ysb1[0:R, 0:cw])
                nc.vector.tensor_add(ysum[64:64 + R, 0:cw],
                                     ysb0[64:64 + R, 0:cw],
                                     ysb1[64:64 + R, 0:cw])
                # far field: out_I += U_d @ Y[I-d], batched over I-quads
                for d in range(1, NB):
                    uc = (d - 1) * 64
                    for q in range(4):
                        I0, I1 = max(d, 4 * q), 4 * q + 3
                        if I0 > I1:
                            continue
                        n = (I1 - I0 + 1) * OS
                        oc = (I0 - 4 * q) * OS
                        jc = (I0 - d) * OS
                        sp = (d == I1)
                        nc.tensor.matmul(
                            pso[0:64, q, oc:oc + n],
                            u[0:R, uc:uc + 64],
                            ysum[0:R, jc:jc + n],
                            start=False, stop=sp, tile_position=(0, 0),
                        )
                        nc.tensor.matmul(
                            pso[64:128, q, oc:oc + n],
                            u[64:64 + R, uc:uc + 64],
                            ysum[64:64 + R, jc:jc + n],
                            start=False, stop=sp, tile_position=(64, 64),
                        )

            for q in range(4):
                psq = pso[:, q, :].rearrange("p (i o) -> p i o", i=4, o=OS)
                if kh == 0:
                    nc.vector.tensor_copy(outacc[:, 4 * q:4 * q + 4, :], psq)
                else:
                    nc.vector.tensor_add(outacc[:, 4 * q:4 * q + 4, :],
                                         outacc[:, 4 * q:4 * q + 4, :], psq)
                    c0 = 4 * q * OS
                    nc.sync.dma_start(
                        out=od_even[:, c0:c0 + 4 * OS],
                        in_=outacc[0:64, 4 * q:4 * q + 4, :])
                    nc.sync.dma_start(
                        out=od_odd[:, c0:c0 + 4 * OS],
                        in_=outacc[64:128, 4 * q:4 * q + 4, :])


_perm = np.concatenate([2 * np.arange(64), 2 * np.arange(64) + 1])  # [128]


def _Tblk(phik, d, par):
    """[64 m, K_USE*128 (k,pp)] : phi[d*128 + 2m+par - perm[pp], k]."""
    idx = d * 128 + 2 * np.arange(64)[:, None] + par - _perm[None, :]
    valid = idx >= 0
    M = np.zeros((64, K_USE, 128))
    for j in range(K_USE):
        Mk = np.zeros((64, 128))
        Mk[valid] = phik[idx[valid], j]
        M[:, j, :] = Mk
    return M.reshape(64, K_USE * 128)


def _build_factors(phik):
    """T0/W/U host factors from kept filters phik [L, K_USE] (float64)."""
    T0 = {par: _Tblk(phik, 0, par) for par in (0, 1)}
    U, W = {}, {}
    for par in (0, 1):
        G = np.concatenate([_Tblk(phik, d, par) for d in range(1, NB)], axis=0)
        _, _, Vt = np.linalg.svd(G, full_matrices=False)
        Wp = Vt[:R].T                                    # [K_USE*128, R]
        W[par] = Wp
        U[par] = [_Tblk(phik, d, par) @ Wp for d in range(1, NB)]
    return T0, W, U


def _prep_inputs(x, phi, M_phi_plus, M_phi_minus):
    """Host-side shard prep. Returns list of 8 input dicts (cores = b*4 + oq)."""
    kidx = np.arange(K - K_USE, K)                       # keep largest sigma
    phik = np.asarray(phi, dtype=np.float64)[:, kidx]

    # xt[p, dc, J*128 + pp] = x[b, J*128 + perm[pp], dc*128+p]
    xts = []
    for b in range(B):
        xb = x[b].reshape(NB, P, D)[:, _perm, :].reshape(L, D)
        xts.append(np.ascontiguousarray(
            xb.T.reshape(4, P, L).transpose(1, 0, 2)).astype(np.float16))

    # mx[p, dc, k*256 + s*128 + oo] = M_s[k, dc*128+p, oq*128+oo]
    mcat = np.stack([M_phi_plus[kidx] + M_phi_minus[kidx],
                     M_phi_plus[kidx] - M_phi_minus[kidx]], axis=1)
    mxs = []
    for oq in range(NOQ):
        m = mcat[:, :, :, oq * OS:(oq + 1) * OS]         # [ku, 2, D, OS]
        m = m.transpose(2, 0, 1, 3).reshape(D, K_USE * 2 * OS)
        mxs.append(np.ascontiguousarray(
            m.reshape(4, P, K_USE * 2 * OS).transpose(1, 0, 2)).astype(np.float16))

    T0, W, U = _build_factors(phik)
    t0h = np.zeros((P, K_USE * P), np.float32)
    for k in range(K_USE):
        for par in (0, 1):
            # t0h[pp, k*128 + par*64 + m] = T0[par][m, k*128+pp]
            t0h[:, k * P + par * 64:k * P + par * 64 + 64] = \
                T0[par][:, k * P:(k + 1) * P].T
    wh = np.zeros((P, K_USE * 2 * R), np.float32)
    for k in range(K_USE):
        for par in (0, 1):
            wh[:, k * 2 * R + par * R:k * 2 * R + (par + 1) * R] = \
                W[par][k * P:(k + 1) * P, :]
    uh = np.zeros((P, (NB - 1) * 64), np.float32)
    for d in range(1, NB):
        uh[0:R, (d - 1) * 64:d * 64] = U[0][d - 1].T
        uh[64:64 + R, (d - 1) * 64:d * 64] = U[1][d - 1].T
    t0h = t0h.astype(np.float16)
    wh = wh.astype(np.float16)
    uh = uh.astype(np.float16)

    in_maps = []
    for b in range(B):
        for oq in range(NOQ):
            in_maps.append({"xt": xts[b], "mx": mxs[oq],
                            "t0": t0h, "w": wh, "u": uh})
    return in_maps


def kernel(x, phi, M_phi_plus, M_phi_minus):
    from concourse.bass_utils import run_bass_kernel_spmd

    x = np.asarray(x, dtype=np.float32)
    phi = np.asarray(phi, dtype=np.float32)
    M_phi_plus = np.asarray(M_phi_plus, dtype=np.float32)
    M_phi_minus = np.asarray(M_phi_minus, dtype=np.float32)

    if "nc" not in _cache:
        _cache["nc"] = _build_bass()
    nc = _cache["nc"]

    in_maps = _prep_inputs(x, phi, M_phi_plus, M_phi_minus)
    results = run_bass_kernel_spmd(nc, in_maps, core_ids=list(range(N_CORES)))

    out = np.empty((B, L, O), dtype=np.float32)
    for c in range(N_CORES):
        b, oq = divmod(c, NOQ)
        r = results.results[c]["out"]                   # [P, NB*OS]
        blk = r.reshape(P, NB, OS).transpose(1, 0, 2).reshape(L, OS)
        out[b, :, oq * OS:(oq + 1) * OS] = blk
    return out


# revision 5
# speedup vs baseline: 1.1774x; 1.0678x over previous
"""MiniSTU Trainium2 kernel — low-rank far-field formulation.

out = T @ (x @ Mp) + sgn (T @ (sgn (x @ Mm))), T block-lower-triangular
Toeplitz from phi.  Polyphase: even output rows need (T@C)_even, odd rows
(T@D)_odd with C/D = x @ (Mp±Mm) interleaved by row parity (stage 1).

Stage 2 splits into:
  - d0: exact dense within-block conv (block distance 0), per filter.
  - far field (block distance d>=1): all 15 block matrices, jointly over
    all filters, share a common rank-R right-singular basis W per output
    parity (numerically R=16 captures 1e-4).  So: Y[J] = W^T B_J (one
    projection per l-block, k-contraction via PSUM accumulation), then
    out_I += sum_d U_d @ Y[I-d] with tiny rank-R matmuls.

This cuts stage-2 PE work ~3.4x vs dense block conv.  8 cores =
batch(2) x output-quarter(4), no collectives; fp16 operands, fp32 PSUM.
"""

import numpy as np

B, L, D, O, K, P = 2, 2048, 512, 512, 16, 128
K_USE = 16        # filters kept (largest sigma); 12 passes at rel err 1.49e-2
R = 32            # shared far-field basis rank per parity (<=32 for tile_position)
NB = L // P       # 16 l-blocks
KH = 2            # k groups (SBUF halving)
KPH = K_USE // KH
NOQ = 4           # o-quarters
OS = O // NOQ     # 128 per-core o slice
CH = KPH * 2 * OS
N_CORES = 8

_cache = {}


def _build_bass(reps=1):
    import contextlib
    import concourse.mybir as mybir
    import concourse.tile as tile
    from concourse import bacc

    dt = mybir.dt
    f16, f32 = dt.float16, dt.float32

    nc = bacc.Bacc("TRN2", target_bir_lowering=False, debug=False,
                   num_devices=N_CORES)

    xt_d = nc.dram_tensor("xt", [P, 4, L], f16, kind="ExternalInput")
    mx_d = nc.dram_tensor("mx", [P, 4, K_USE * 2 * OS], f16, kind="ExternalInput")
    t0_d = nc.dram_tensor("t0", [P, K_USE * P], f16, kind="ExternalInput")
    w_d = nc.dram_tensor("w", [P, K_USE * 2 * R], f16, kind="ExternalInput")
    u_d = nc.dram_tensor("u", [P, (NB - 1) * 64], f16, kind="ExternalInput")
    out_d = nc.dram_tensor("out", [P, NB * OS], f32, kind="ExternalOutput")

    with tile.TileContext(nc) as tc:
        with (
            tc.tile_pool(name="const", bufs=1) as cpool,
            tc.tile_pool(name="apool", bufs=1) as apool,
            tc.tile_pool(name="ypool", bufs=1) as ypool,
            tc.tile_pool(name="opool", bufs=1) as opool,
        ):
            xt = cpool.tile([P, 4, L], f16, tag="xt")
            mx = cpool.tile([P, 4, K_USE * 2 * OS], f16, tag="mx")
            t0 = cpool.tile([P, K_USE * P], f16, tag="t0")
            w = cpool.tile([P, K_USE * 2 * R], f16, tag="w")
            u = cpool.tile([P, (NB - 1) * 64], f16, tag="u")
            a_ev = apool.tile([P, NB, KPH * OS], f16, tag="aev")
            a_od = apool.tile([P, NB, KPH * OS], f16, tag="aod")
            ysb0 = ypool.tile([P, NB * OS], f16, tag="ysb0")
            ysb1 = ypool.tile([P, NB * OS], f16, tag="ysb1")
            ysum = ypool.tile([P, NB * OS], f16, tag="ysum")
            outacc = opool.tile([P, NB, OS], f32, tag="outacc")

            for dc in range(4):
                nc.sync.dma_start(out=xt[:, dc, :], in_=xt_d[:, dc, :])
                nc.sync.dma_start(out=mx[:, dc, :], in_=mx_d[:, dc, :])
            nc.sync.dma_start(out=t0[:], in_=t0_d[:])
            nc.sync.dma_start(out=w[:], in_=w_d[:])
            nc.sync.dma_start(out=u[:], in_=u_d[:])

            loop_cm = (tc.For_i(0, reps, 1,
                                hint_engines=(mybir.EngineType.PE,
                                              mybir.EngineType.DVE))
                       if reps > 1 else contextlib.nullcontext())
            with loop_cm:
                _emit_body(nc, tc, mybir, f16, f32, xt, mx, t0, w, u,
                           a_ev, a_od, ysb0, ysb1, ysum, outacc, out_d)

    nc.compile()
    return nc


def _emit_body(nc, tc, mybir, f16, f32, xt, mx, t0, w, u,
               a_ev, a_od, ysb0, ysb1, ysum, outacc, out_d):
    od_even = out_d[:].rearrange("(h two) c -> two h c", two=2)[0]
    od_odd = out_d[:].rearrange("(h two) c -> two h c", two=2)[1]

    SH = KPH * OS                       # columns per s-half of a k-group
    for kh in range(KH):
        # ---- stage 1: parity-split col-tiled pairs.  Even-l rows (array
        # cols 0-63) and odd-l rows (cols 64-127) run concurrently with
        # their own M stream, so psA == a_ev and psB == a_od land in
        # final layout and drain as single full-128-partition copies.
        with tc.tile_pool(name="ps1", bufs=2, space="PSUM") as ps1pool:
            for J in range(NB):
                psA = ps1pool.tile([P, SH], f32, tag="psA")
                psB = ps1pool.tile([P, SH], f32, tag="psB")
                for dc in range(4):
                    xtE = xt[:, dc, J * P:J * P + 64]
                    xtO = xt[:, dc, J * P + 64:(J + 1) * P]
                    for n in range(SH // 512):
                        c = n * 512
                        mS = mx[:, dc, kh * CH + c: kh * CH + c + 512]
                        mD = mx[:, dc, kh * CH + SH + c: kh * CH + SH + c + 512]
                        st, sp = (dc == 0), (dc == 3)
                        nc.tensor.matmul(psA[0:64, c:c + 512], xtE, mS,
                                         start=st, stop=sp, tile_position=(0, 0))
                        nc.tensor.matmul(psA[64:128, c:c + 512], xtO, mD,
                                         start=st, stop=sp, tile_position=(0, 64))
                        nc.tensor.matmul(psB[0:64, c:c + 512], xtE, mD,
                                         start=st, stop=sp, tile_position=(0, 0))
                        nc.tensor.matmul(psB[64:128, c:c + 512], xtO, mS,
                                         start=st, stop=sp, tile_position=(0, 64))
                nc.vector.tensor_copy(a_ev[:, J, :], psA[:])
                nc.scalar.copy(a_od[:, J, :], psB[:])

        # ---- stage 2: pso = d0 (exact diagonal blocks) [+ far field at
        # kh=1]; psy = rank-R far-field projections Y[J] = W^T B_J.
        with (
            tc.tile_pool(name="ps2o", bufs=1, space="PSUM") as psopool,
            tc.tile_pool(name="ps2y", bufs=1, space="PSUM") as psypool,
        ):
            pso = psopool.tile([P, 4, 512], f32, tag="pso")
            psy = psypool.tile([P, 4, 512], f32, tag="psy")

            for kl in range(KPH):
                tc0 = (kh * KPH + kl) * P
                for q in range(4):
                    st = (kl == 0)
                    sp = (kh == 0 and kl == KPH - 1)
                    nc.tensor.matmul(
                        pso[0:64, q, :],
                        t0[:, tc0:tc0 + 64],
                        a_ev[:, 4 * q:4 * q + 4, kl * OS:(kl + 1) * OS],
                        start=st, stop=sp, tile_position=(0, 0),
                    )
                    nc.tensor.matmul(
                        pso[64:128, q, :],
                        t0[:, tc0 + 64:tc0 + P],
                        a_od[:, 4 * q:4 * q + 4, kl * OS:(kl + 1) * OS],
                        start=st, stop=sp, tile_position=(0, 64),
                    )

            for kl in range(KPH):
                wc = (kh * KPH + kl) * 2 * R
                for q in range(4):
                    jn = 4 if q < 3 else 3      # Y[15] is never used
                    st = (kl == 0)
                    sp = (kl == KPH - 1)
                    nc.tensor.matmul(
                        psy[0:R, q, 0:jn * OS],
                        w[:, wc:wc + R],
                        a_ev[:, 4 * q:4 * q + jn, kl * OS:(kl + 1) * OS],
                        start=st, stop=sp, tile_position=(0, 0),
                    )
                    nc.tensor.matmul(
                        psy[64:64 + R, q, 0:jn * OS],
                        w[:, wc + R:wc + 2 * R],
                        a_od[:, 4 * q:4 * q + jn, kl * OS:(kl + 1) * OS],
                        start=st, stop=sp, tile_position=(0, 64),
                    )

            ysb = ysb0 if kh == 0 else ysb1
            for q in range(4):
                jn = 4 if q < 3 else 3
                c0, cw = 4 * q * OS, jn * OS
                nc.vector.tensor_copy(ysb[0:R, c0:c0 + cw], psy[0:R, q, 0:cw])
                nc.vector.tensor_copy(ysb[64:64 + R, c0:c0 + cw],
                                      psy[64:64 + R, q, 0:cw])

            if kh == KH - 1:
                cw = (NB - 1) * OS
                nc.vector.tensor_add(ysum[0:R, 0:cw], ysb0[0:R, 0:cw],
                                     ysb1[0:R, 0:cw])
                nc.vector.tensor_add(ysum[64:64 + R, 0:cw],
                                     ysb0[64:64 + R, 0:cw],
                                     ysb1[64:64 + R, 0:cw])
                # far field: out_I += U_d @ Y[I-d], batched over I-quads
                for d in range(1, NB):
                    uc = (d - 1) * 64
                    for q in range(4):
                        I0, I1 = max(d, 4 * q), 4 * q + 3
                        if I0 > I1:
                            continue
                        n = (I1 - I0 + 1) * OS
                        oc = (I0 - 4 * q) * OS
                        jc = (I0 - d) * OS
                        sp = (d == I1)
                        nc.tensor.matmul(
                            pso[0:64, q, oc:oc + n],
                            u[0:R, uc:uc + 64],
                            ysum[0:R, jc:jc + n],
                            start=False, stop=sp, tile_position=(0, 0),
                        )
                        nc.tensor.matmul(
                            pso[64:128, q, oc:oc + n],
                            u[64:64 + R, uc:uc + 64],
                            ysum[64:64 + R, jc:jc + n],
                            start=False, stop=sp, tile_position=(64, 64),
                        )

            for q in range(4):
                psq = pso[:, q, :].rearrange("p (i o) -> p i o", i=4, o=OS)
                if kh == 0:
                    nc.vector.tensor_copy(outacc[:, 4 * q:4 * q + 4, :], psq)
                else:
                    nc.vector.tensor_add(outacc[:, 4 * q:4 * q + 4, :],
                                         outacc[:, 4 * q:4 * q + 4, :], psq)
                    c0 = 4 * q * OS
                    nc.sync.dma_start(
                        out=od_even[:, c0:c0 + 4 * OS],
                        in_=outacc[0:64, 4 * q:4 * q + 4, :])
                    nc.sync.dma_start(
                        out=od_odd[:, c0:c0 + 4 * OS],
                        in_=outacc[64:128, 4 * q:4 * q + 4, :])


_perm = np.concatenate([2 * np.arange(64), 2 * np.arange(64) + 1])  # [128]


def _Tblk(phik, d, par):
    """[64 m, K_USE*128 (k,pp)] : phi[d*128 + 2m+par - perm[pp], k]."""
    idx = d * 128 + 2 * np.arange(64)[:, None] + par - _perm[None, :]
    valid = idx >= 0
    M = np.zeros((64, K_USE, 128))
    for j in range(K_USE):
        Mk = np.zeros((64, 128))
        Mk[valid] = phik[idx[valid], j]
        M[:, j, :] = Mk
    return M.reshape(64, K_USE * 128)


def _build_factors(phik):
    """T0/W/U host factors from kept filters phik [L, K_USE] (float64)."""
    T0 = {par: _Tblk(phik, 0, par) for par in (0, 1)}
    U, W = {}, {}
    for par in (0, 1):
        G = np.concatenate([_Tblk(phik, d, par) for d in range(1, NB)], axis=0)
        _, _, Vt = np.linalg.svd(G, full_matrices=False)
        Wp = Vt[:R].T                                    # [K_USE*128, R]
        W[par] = Wp
        U[par] = [_Tblk(phik, d, par) @ Wp for d in range(1, NB)]
    return T0, W, U


def _prep_inputs(x, phi, M_phi_plus, M_phi_minus):
    """Host-side shard prep. Returns list of 8 input dicts (cores = b*4 + oq)."""
    kidx = np.arange(K - K_USE, K)                       # keep largest sigma
    phik = np.asarray(phi, dtype=np.float64)[:, kidx]

    # xt[p, dc, J*128 + pp] = x[b, J*128 + perm[pp], dc*128+p]
    xts = []
    for b in range(B):
        xb = x[b].reshape(NB, P, D)[:, _perm, :].reshape(L, D)
        xts.append(np.ascontiguousarray(
            xb.T.reshape(4, P, L).transpose(1, 0, 2)).astype(np.float16))

    # mx[p, dc, (kh, s, kl, oo)] = M_s[kh*KPH+kl, dc*128+p, oq*128+oo]
    mcat = np.stack([M_phi_plus[kidx] + M_phi_minus[kidx],
                     M_phi_plus[kidx] - M_phi_minus[kidx]], axis=1)
    mxs = []
    for oq in range(NOQ):
        m = mcat[:, :, :, oq * OS:(oq + 1) * OS]         # [ku, 2, D, OS]
        m = m.reshape(KH, KPH, 2, D, OS).transpose(3, 0, 2, 1, 4)
        m = m.reshape(D, K_USE * 2 * OS)
        mxs.append(np.ascontiguousarray(
            m.reshape(4, P, K_USE * 2 * OS).transpose(1, 0, 2)).astype(np.float16))

    T0, W, U = _build_factors(phik)
    t0h = np.zeros((P, K_USE * P), np.float32)
    for k in range(K_USE):
        for par in (0, 1):
            # t0h[pp, k*128 + par*64 + m] = T0[par][m, k*128+pp]
            t0h[:, k * P + par * 64:k * P + par * 64 + 64] = \
                T0[par][:, k * P:(k + 1) * P].T
    wh = np.zeros((P, K_USE * 2 * R), np.float32)
    for k in range(K_USE):
        for par in (0, 1):
            wh[:, k * 2 * R + par * R:k * 2 * R + (par + 1) * R] = \
                W[par][k * P:(k + 1) * P, :]
    uh = np.zeros((P, (NB - 1) * 64), np.float32)
    for d in range(1, NB):
        uh[0:R, (d - 1) * 64:d * 64] = U[0][d - 1].T
        uh[64:64 + R, (d - 1) * 64:d * 64] = U[1][d - 1].T
    t0h = t0h.astype(np.float16)
    wh = wh.astype(np.float16)
    uh = uh.astype(np.float16)

    in_maps = []
    for b in range(B):
        for oq in range(NOQ):
            in_maps.append({"xt": xts[b], "mx": mxs[oq],
                            "t0": t0h, "w": wh, "u": uh})
    return in_maps


def kernel(x, phi, M_phi_plus, M_phi_minus):
    from concourse.bass_utils import run_bass_kernel_spmd

    x = np.asarray(x, dtype=np.float32)
    phi = np.asarray(phi, dtype=np.float32)
    M_phi_plus = np.asarray(M_phi_plus, dtype=np.float32)
    M_phi_minus = np.asarray(M_phi_minus, dtype=np.float32)

    if "nc" not in _cache:
        _cache["nc"] = _build_bass()
    nc = _cache["nc"]

    in_maps = _prep_inputs(x, phi, M_phi_plus, M_phi_minus)
    results = run_bass_kernel_spmd(nc, in_maps, core_ids=list(range(N_CORES)))

    out = np.empty((B, L, O), dtype=np.float32)
    for c in range(N_CORES):
        b, oq = divmod(c, NOQ)
        r = results.results[c]["out"]                   # [P, NB*OS]
        blk = r.reshape(P, NB, OS).transpose(1, 0, 2).reshape(L, OS)
        out[b, :, oq * OS:(oq + 1) * OS] = blk
    return out
